# revision 7
# baseline (speedup 1.0000x reference)
"""Nearest-neighbor retrieval kernel for Trainium2 (8 NeuronCores, SPMD).

Problem: dis[i] = mean((in_vel - train_obs_vel[i])**2); return
train_target_vel[argmin(dis)].

Strategy: only train_obs_vel (422 MB) has to stream through the devices.
Shard it row-wise across the 8 cores. Each core computes, per local sample i,
the key  k_i = sum((x_i - q)^2)  (= 1056 * dis_i), using:
  - VectorE TENSOR_TENSOR(subtract)      -> diff
  - ScalarE ACTIVATE(Square, accum_out)  -> sum(diff^2) per sample
One pass per engine over the data, so the kernel is DMA/HBM-bound.
The tiny [12500] key vector per core is returned to the host, which does the
global argmin and the 6.6 KB gather from train_target_vel (no device
collectives needed).
"""

import sys

sys.path.insert(0, "/opt/trn_rl_repo")

import numpy as np

import concourse.bacc as bacc
import concourse.mybir as mybir
import concourse.tile as tile
from concourse.bass_utils import run_bass_kernel_spmd

# Problem shapes (hardcoded per harness contract)
N = 100000
T_OBS = 16
T_OUT = 25
D = 66
F = T_OBS * D  # 1056 features per sample
CORES = 8
PER = N // CORES  # 12500 samples per core
P = 125  # partitions used (125 * 100 = 12500)
C = PER // P  # 100 samples (columns) per partition
S = 5  # samples per partition per DMA tile
NTILES = C // S  # 20 DMA tiles per core

_f32 = mybir.dt.float32


def build_nc(s=S, xin_bufs=4):
    ntiles = C // s
    assert ntiles * s == C
    nc = bacc.Bacc("TRN2", target_bir_lowering=False, debug=False)
    x = nc.dram_tensor("x", [PER, F], _f32, kind="ExternalInput")
    qb = nc.dram_tensor("qb", [128, F], _f32, kind="ExternalInput")
    key_out = nc.dram_tensor("key", [P, C], _f32, kind="ExternalOutput")

    # [12500, 1056] -> [125 partitions, 100*1056 contiguous floats]
    xr = x[:].rearrange("(p c) d -> p (c d)", p=P)

    with tile.TileContext(nc) as tc:
        with (
            tc.tile_pool(name="xin", bufs=xin_bufs) as xpool,
            tc.tile_pool(name="qpool", bufs=1) as qpool,
            tc.tile_pool(name="scratch", bufs=2) as spool,
            tc.tile_pool(name="acc", bufs=1) as apool,
        ):
            q_tile = qpool.tile([128, F], _f32)
            nc.sync.dma_start(out=q_tile[:], in_=qb[:])

            key_t = apool.tile([P, C], _f32)

            for t in range(ntiles):
                xt = xpool.tile([P, s * F], _f32, tag="xt")
                nc.sync.dma_start(
                    out=xt[:], in_=xr[:, t * s * F : (t + 1) * s * F]
                )
                for j in range(s):
                    col = t * s + j
                    xs = xt[:, j * F : (j + 1) * F]
                    # diff = x - q on VectorE
                    diff = spool.tile([P, F], _f32, tag="diff")
                    nc.vector.tensor_sub(diff[:], xs, q_tile[0:P, :])
                    # key = sum(diff^2) on ScalarE (Square + free-axis accum)
                    sq_scr = spool.tile([P, F], _f32, tag="sq")
                    nc.scalar.activation(
                        out=sq_scr[:],
                        in_=diff[:],
                        func=mybir.ActivationFunctionType.Square,
                        accum_out=key_t[:, col : col + 1],
                    )

            nc.sync.dma_start(out=key_out[:], in_=key_t[:])
    nc.compile()
    return nc


_nc_cache = {}


def _get_nc():
    key = (S,)
    if key not in _nc_cache:
        _nc_cache[key] = build_nc()
    return _nc_cache[key]


def make_in_maps(in_vel, train_obs_vel):
    q = np.ascontiguousarray(np.asarray(in_vel, dtype=np.float32).reshape(F))
    qb = np.ascontiguousarray(np.broadcast_to(q, (128, F)))
    X = np.asarray(train_obs_vel, dtype=np.float32).reshape(N, F)
    return [
        {"x": np.ascontiguousarray(X[c * PER : (c + 1) * PER]), "qb": qb}
        for c in range(CORES)
    ]


def finish(results, train_target_vel):
    # keys[core][p, col] is the key of global sample core*PER + p*C + col;
    # flattening in C order reproduces exactly the original sample order.
    keys = np.stack([np.asarray(r["key"]) for r in results])
    best = int(keys.reshape(-1).argmin())
    out = np.asarray(train_target_vel)[best]
    return np.ascontiguousarray(out)


def kernel(in_vel, train_obs_vel, train_target_vel):
    nc = _get_nc()
    in_maps = make_in_maps(in_vel, train_obs_vel)
    res = run_bass_kernel_spmd(nc, in_maps, list(range(CORES)))
    return finish(res.results, train_target_vel)


# revision 8
# speedup vs baseline: 2.2958x; 2.2958x over previous
"""Nearest-neighbor retrieval kernel for Trainium2 (8 NeuronCores, SPMD).

Problem: dis[i] = mean((in_vel - train_obs_vel[i])**2); return
train_target_vel[argmin(dis)].

Strategy: only train_obs_vel (422 MB) has to stream through the devices.
Shard it row-wise across the 8 cores (12500 rows each, padded to
12544 = 128*98 so DMA tiles have 128 partitions — the HWDGE only engages
all 16 SDMA engines for 128-partition descriptors lists). Each core
computes, per local sample i, the key  k_i = sum((x_i - q)^2)
(= 1056 * dis_i), using:
  - VectorE TENSOR_TENSOR(subtract)      -> diff
  - ScalarE ACTIVATE(Square, accum_out)  -> sum(diff^2) per sample
One pass per engine over the data, so the kernel is DMA/HBM-bound
(~53 MB/core at ~370 GB/s).
The tiny [128, 98] key tile per core returns to the host, which does the
global argmin and the 6.6 KB gather from train_target_vel (no device
collectives needed).
"""

import sys

sys.path.insert(0, "/opt/trn_rl_repo")

import numpy as np

import concourse.bacc as bacc
import concourse.mybir as mybir
import concourse.tile as tile
from concourse.bass_utils import run_bass_kernel_spmd

# Problem shapes (hardcoded per harness contract)
N = 100000
T_OBS = 16
T_OUT = 25
D = 66
F = T_OBS * D  # 1056 features per sample
CORES = 8
PER = N // CORES  # 12500 samples per core
P = 128  # SBUF partitions
C = 98  # samples (columns) per partition
PER_PAD = P * C  # 12544 padded samples per core
PAD_VAL = 1.0e4  # pad rows get a huge distance; never the argmin
S = 7  # samples per partition per DMA tile
NTILES = C // S  # 14 DMA tiles per core

_f32 = mybir.dt.float32


def build_nc(s=S, xin_bufs=4):
    ntiles = C // s
    assert ntiles * s == C
    nc = bacc.Bacc("TRN2", target_bir_lowering=False, debug=False)
    x = nc.dram_tensor("x", [PER_PAD, F], _f32, kind="ExternalInput")
    qb = nc.dram_tensor("qb", [P, F], _f32, kind="ExternalInput")
    key_out = nc.dram_tensor("key", [P, C], _f32, kind="ExternalOutput")

    # [12544, 1056] -> [128 partitions, 98*1056 contiguous floats]
    xr = x[:].rearrange("(p c) d -> p (c d)", p=P)

    with tile.TileContext(nc) as tc:
        with (
            tc.tile_pool(name="xin", bufs=xin_bufs) as xpool,
            tc.tile_pool(name="qpool", bufs=1) as qpool,
            tc.tile_pool(name="scratch", bufs=2) as spool,
            tc.tile_pool(name="acc", bufs=1) as apool,
        ):
            q_tile = qpool.tile([P, F], _f32)
            nc.sync.dma_start(out=q_tile[:], in_=qb[:])

            key_t = apool.tile([P, C], _f32)

            for t in range(ntiles):
                xt = xpool.tile([P, s * F], _f32, tag="xt")
                nc.sync.dma_start(
                    out=xt[:], in_=xr[:, t * s * F : (t + 1) * s * F]
                )
                for j in range(s):
                    col = t * s + j
                    xs = xt[:, j * F : (j + 1) * F]
                    # diff = x - q on VectorE
                    diff = spool.tile([P, F], _f32, tag="diff")
                    nc.vector.tensor_sub(diff[:], xs, q_tile[:])
                    # key = sum(diff^2) on ScalarE (Square + free-axis accum)
                    sq_scr = spool.tile([P, F], _f32, tag="sq")
                    nc.scalar.activation(
                        out=sq_scr[:],
                        in_=diff[:],
                        func=mybir.ActivationFunctionType.Square,
                        accum_out=key_t[:, col : col + 1],
                    )

            nc.sync.dma_start(out=key_out[:], in_=key_t[:])
    nc.compile()
    return nc


_nc_cache = {}


def _get_nc():
    key = (S,)
    if key not in _nc_cache:
        _nc_cache[key] = build_nc()
    return _nc_cache[key]


def make_in_maps(in_vel, train_obs_vel):
    q = np.ascontiguousarray(np.asarray(in_vel, dtype=np.float32).reshape(F))
    qbn = np.ascontiguousarray(np.broadcast_to(q, (P, F)))
    X = np.asarray(train_obs_vel, dtype=np.float32).reshape(N, F)
    in_maps = []
    for c in range(CORES):
        xp = np.full((PER_PAD, F), PAD_VAL, dtype=np.float32)
        xp[:PER] = X[c * PER : (c + 1) * PER]
        in_maps.append({"x": xp, "qb": qbn})
    return in_maps


def finish(results, train_target_vel):
    # keys[core][p, col] is the key of padded-local sample p*C + col;
    # flattening in C order reproduces the padded-local sample order.
    keys = np.stack([np.asarray(r["key"]) for r in results])  # [8, P, C]
    flat = keys.reshape(CORES, PER_PAD)[:, :PER]  # drop pad rows
    best = int(flat.reshape(-1).argmin())  # global index, core-major
    out = np.asarray(train_target_vel)[best]
    return np.ascontiguousarray(out)


def kernel(in_vel, train_obs_vel, train_target_vel):
    nc = _get_nc()
    in_maps = make_in_maps(in_vel, train_obs_vel)
    res = run_bass_kernel_spmd(nc, in_maps, list(range(CORES)))
    return finish(res.results, train_target_vel)


# revision 9
# speedup vs baseline: 2.8770x; 1.2532x over previous
"""Nearest-neighbor retrieval kernel for Trainium2 (8 NeuronCores, SPMD).

Problem: dis[i] = mean((in_vel - train_obs_vel[i])**2); return
train_target_vel[argmin(dis)].

Strategy: only train_obs_vel (422 MB) has to stream through the devices.
Shard it row-wise across the 8 cores (12500 rows each, padded to
12544 = 128*98 so DMA tiles have 128 partitions — the HWDGE only engages
all 16 SDMA engines for 128-partition descriptors lists). Each core
computes, per local sample i, the key  k_i = sum((x_i - q)^2)
(= 1056 * dis_i), using:
  - VectorE TENSOR_TENSOR(subtract)      -> diff
  - ScalarE ACTIVATE(Square, accum_out)  -> sum(diff^2) per sample
One pass per engine over the data, so the kernel is DMA/HBM-bound
(~53 MB/core at ~370 GB/s).
The tiny [128, 98] key tile per core returns to the host, which does the
global argmin and the 6.6 KB gather from train_target_vel (no device
collectives needed).
"""

import sys

sys.path.insert(0, "/opt/trn_rl_repo")

import numpy as np

import concourse.bacc as bacc
import concourse.mybir as mybir
import concourse.tile as tile
from concourse.bass_utils import run_bass_kernel_spmd

# Problem shapes (hardcoded per harness contract)
N = 100000
T_OBS = 16
T_OUT = 25
D = 66
F = T_OBS * D  # 1056 features per sample
CORES = 8
PER = N // CORES  # 12500 samples per core
P = 128  # SBUF partitions
C = 98  # samples (columns) per partition
PER_PAD = P * C  # 12544 padded samples per core
PAD_VAL = 1.0e4  # pad rows get a huge distance; never the argmin
S = 2  # samples per partition per DMA tile
NTILES = C // S  # DMA tiles per core

_f32 = mybir.dt.float32


def build_nc(s=S, xin_bufs=8):
    ntiles = C // s
    assert ntiles * s == C
    nc = bacc.Bacc("TRN2", target_bir_lowering=False, debug=False)
    x = nc.dram_tensor("x", [PER_PAD, F], _f32, kind="ExternalInput")
    qb = nc.dram_tensor("qb", [P, F], _f32, kind="ExternalInput")
    key_out = nc.dram_tensor("key", [P, C], _f32, kind="ExternalOutput")

    # [12544, 1056] -> [128 partitions, 98*1056 contiguous floats]
    xr = x[:].rearrange("(p c) d -> p (c d)", p=P)

    with tile.TileContext(nc) as tc:
        with (
            tc.tile_pool(name="xin", bufs=xin_bufs) as xpool,
            tc.tile_pool(name="qpool", bufs=1) as qpool,
            tc.tile_pool(name="scratch", bufs=4) as spool,
            tc.tile_pool(name="acc", bufs=1) as apool,
        ):
            q_tile = qpool.tile([P, F], _f32)
            nc.sync.dma_start(out=q_tile[:], in_=qb[:])

            key_t = apool.tile([P, C], _f32)

            for t in range(ntiles):
                xt = xpool.tile([P, s * F], _f32, tag="xt")
                nc.sync.dma_start(
                    out=xt[:], in_=xr[:, t * s * F : (t + 1) * s * F]
                )
                for j in range(s):
                    col = t * s + j
                    xs = xt[:, j * F : (j + 1) * F]
                    # diff = x - q on VectorE
                    diff = spool.tile([P, F], _f32, tag="diff")
                    nc.vector.tensor_sub(diff[:], xs, q_tile[:])
                    # key = sum(diff^2) on ScalarE (Square + free-axis accum)
                    sq_scr = spool.tile([P, F], _f32, tag="sq")
                    nc.scalar.activation(
                        out=sq_scr[:],
                        in_=diff[:],
                        func=mybir.ActivationFunctionType.Square,
                        accum_out=key_t[:, col : col + 1],
                    )

            nc.sync.dma_start(out=key_out[:], in_=key_t[:])
    nc.compile()
    return nc


_nc_cache = {}


def _get_nc():
    key = (S,)
    if key not in _nc_cache:
        _nc_cache[key] = build_nc()
    return _nc_cache[key]


def make_in_maps(in_vel, train_obs_vel):
    q = np.ascontiguousarray(np.asarray(in_vel, dtype=np.float32).reshape(F))
    qbn = np.ascontiguousarray(np.broadcast_to(q, (P, F)))
    X = np.asarray(train_obs_vel, dtype=np.float32).reshape(N, F)
    in_maps = []
    for c in range(CORES):
        xp = np.full((PER_PAD, F), PAD_VAL, dtype=np.float32)
        xp[:PER] = X[c * PER : (c + 1) * PER]
        in_maps.append({"x": xp, "qb": qbn})
    return in_maps


def finish(results, train_target_vel):
    # keys[core][p, col] is the key of padded-local sample p*C + col;
    # flattening in C order reproduces the padded-local sample order.
    keys = np.stack([np.asarray(r["key"]) for r in results])  # [8, P, C]
    flat = keys.reshape(CORES, PER_PAD)[:, :PER]  # drop pad rows
    best = int(flat.reshape(-1).argmin())  # global index, core-major
    out = np.asarray(train_target_vel)[best]
    return np.ascontiguousarray(out)


def kernel(in_vel, train_obs_vel, train_target_vel):
    nc = _get_nc()
    in_maps = make_in_maps(in_vel, train_obs_vel)
    res = run_bass_kernel_spmd(nc, in_maps, list(range(CORES)))
    return finish(res.results, train_target_vel)


# revision 10
# speedup vs baseline: 3.6126x; 1.2557x over previous
"""Nearest-neighbor retrieval kernel for Trainium2 (8 NeuronCores, SPMD).

Problem: dis[i] = mean((in_vel - train_obs_vel[i])**2); return
train_target_vel[argmin(dis)].

Strategy: only train_obs_vel has to stream through the devices. The device
pass is a bf16 *screen*: it computes approximate keys
k_i ~= sum((x_i - q)^2) for every sample (bf16 halves HBM traffic to
~26.5 MB/core and doubles VectorE throughput). The host then recomputes
exact f32 keys for the top-1024 screened candidates (~1 M flops) and picks
the true argmin — bf16 key noise is ~+-0.3 on a min-gap of ~4, so the true
argmin is inside the top-1024 with overwhelming margin, and the final
result is bit-exact.

Sharding: 12500 rows per core, padded to 12544 = 128*98 so DMA tiles have
128 partitions (the HWDGE only engages all 16 SDMA engines for
128-partition descriptor lists; at 125 partitions it uses 5 and runs 3x
slower). Per column (sample-slice) the engines split work:
  - VectorE  TENSOR_TENSOR(subtract) bf16 (2x mode)   -> diff
  - ScalarE  ACTIVATE(Square, accum_out)              -> key (most cols)
  - VectorE  TENSOR_TENSOR(mult) + TENSOR_REDUCE(add) -> key (offload cols)
so ScalarE (dtype-independent 1 elem/cycle) stops being the bottleneck.
The tiny [128, 98] f32 key tile per core returns to the host; no device
collectives are needed.
"""

import sys

sys.path.insert(0, "/opt/trn_rl_repo")

import ml_dtypes
import numpy as np

import concourse.bacc as bacc
import concourse.mybir as mybir
import concourse.tile as tile
from concourse.bass_utils import run_bass_kernel_spmd

# Problem shapes (hardcoded per harness contract)
N = 100000
T_OBS = 16
T_OUT = 25
D = 66
F = T_OBS * D  # 1056 features per sample
CORES = 8
PER = N // CORES  # 12500 samples per core
P = 128  # SBUF partitions
C = 98  # samples (columns) per partition
PER_PAD = P * C  # 12544 padded samples per core
PAD_VAL = 1.0e4  # pad rows get a huge distance; never the argmin
S = 2  # samples per partition per DMA tile
VEC_EVERY = 5  # every 5th column: square+reduce on VectorE instead of ScalarE
TOPK = 1024  # host-side exact recheck pool

_f32 = mybir.dt.float32
_bf16 = mybir.dt.bfloat16
_bf16_np = ml_dtypes.bfloat16


def build_nc(s=S, xin_bufs=8, vec_every=VEC_EVERY):
    ntiles = C // s
    assert ntiles * s == C
    nc = bacc.Bacc("TRN2", target_bir_lowering=False, debug=False)
    x = nc.dram_tensor("x", [PER_PAD, F], _bf16, kind="ExternalInput")
    qb = nc.dram_tensor("qb", [P, F], _bf16, kind="ExternalInput")
    key_out = nc.dram_tensor("key", [P, C], _f32, kind="ExternalOutput")

    # [12544, 1056] -> [128 partitions, 98*1056 contiguous bf16]
    xr = x[:].rearrange("(p c) d -> p (c d)", p=P)

    with tile.TileContext(nc) as tc:
        with (
            tc.tile_pool(name="xin", bufs=xin_bufs) as xpool,
            tc.tile_pool(name="qpool", bufs=1) as qpool,
            tc.tile_pool(name="scratch", bufs=4) as spool,
            tc.tile_pool(name="acc", bufs=1) as apool,
        ):
            q_tile = qpool.tile([P, F], _bf16)
            nc.sync.dma_start(out=q_tile[:], in_=qb[:])

            key_t = apool.tile([P, C], _f32)

            for t in range(ntiles):
                xt = xpool.tile([P, s * F], _bf16, tag="xt")
                nc.sync.dma_start(
                    out=xt[:], in_=xr[:, t * s * F : (t + 1) * s * F]
                )
                for j in range(s):
                    col = t * s + j
                    xs = xt[:, j * F : (j + 1) * F]
                    diff = spool.tile([P, F], _bf16, tag="diff")
                    nc.vector.tensor_sub(diff[:], xs, q_tile[:])
                    kcol = key_t[:, col : col + 1]
                    if col % vec_every == vec_every - 1:
                        # VectorE path: mult + reduce
                        sq = spool.tile([P, F], _bf16, tag="vsq")
                        nc.vector.tensor_mul(sq[:], diff[:], diff[:])
                        nc.vector.tensor_reduce(
                            kcol,
                            sq[:],
                            axis=mybir.AxisListType.X,
                            op=mybir.AluOpType.add,
                        )
                    else:
                        # ScalarE path: Square with free-axis accumulate
                        sq = spool.tile([P, F], _bf16, tag="ssq")
                        nc.scalar.activation(
                            out=sq[:],
                            in_=diff[:],
                            func=mybir.ActivationFunctionType.Square,
                            accum_out=kcol,
                        )

            nc.sync.dma_start(out=key_out[:], in_=key_t[:])
    nc.compile()
    return nc


_nc_cache = {}


def _get_nc():
    key = (S, VEC_EVERY)
    if key not in _nc_cache:
        _nc_cache[key] = build_nc()
    return _nc_cache[key]


def make_in_maps(in_vel, train_obs_vel):
    q = np.asarray(in_vel, dtype=np.float32).reshape(F)
    qbn = np.ascontiguousarray(
        np.broadcast_to(q.astype(_bf16_np), (P, F))
    )
    X = np.asarray(train_obs_vel, dtype=np.float32).reshape(N, F)
    Xb = X.astype(_bf16_np)
    in_maps = []
    for c in range(CORES):
        xp = np.full((PER_PAD, F), PAD_VAL, dtype=_bf16_np)
        xp[:PER] = Xb[c * PER : (c + 1) * PER]
        in_maps.append({"x": xp, "qb": qbn})
    return in_maps


def finish(results, in_vel, train_obs_vel, train_target_vel):
    # keys[core][p, col] screens padded-local sample p*C + col; flattening
    # in C order reproduces the padded-local sample order.
    keys = np.stack([np.asarray(r["key"]) for r in results])  # [8, P, C]
    flat = keys.reshape(CORES, PER_PAD)[:, :PER].reshape(-1)  # drop pads
    k = min(TOPK, flat.size)
    cand = np.sort(np.argpartition(flat, k - 1)[:k])
    # exact f32 recheck of the screened candidates
    q = np.asarray(in_vel, dtype=np.float32).reshape(F)
    X = np.asarray(train_obs_vel, dtype=np.float32).reshape(N, F)
    d = X[cand] - q
    exact = np.einsum("ij,ij->i", d, d)
    best = int(cand[int(exact.argmin())])
    out = np.asarray(train_target_vel)[best]
    return np.ascontiguousarray(out)


def kernel(in_vel, train_obs_vel, train_target_vel):
    nc = _get_nc()
    in_maps = make_in_maps(in_vel, train_obs_vel)
    res = run_bass_kernel_spmd(nc, in_maps, list(range(CORES)))
    return finish(res.results, in_vel, train_obs_vel, train_target_vel)
